# revision 3
# baseline (speedup 1.0000x reference)
"""BERT self-attention (B=4, S=2048, H=1024, 16 heads x 64) on 8 TRN2 NeuronCores.

Sharding: data-parallel over batch (4) x tensor-parallel over head-groups (2).
Core c handles batch c//2 and heads [8*(c%2), 8*(c%2)+8): it gets the full
hidden_states[b] plus the 512 W-columns/bias entries for its heads, and
produces out[b, :, 512*g : 512*(g+1)]. No cross-core communication.

Per-core kernel (all matmuls bf16, f32 accumulation in PSUM):
  xT   = transpose(x) via PE                      [1024h, 2048s]
  QT/KT = W.T @ xT  (+bias)                       [512hd, 2048s]
  V'   = xT.T @ Wv (+bias), 65-col per head with an appended ones column
  per (head-pair, q-macro 512, k-chunk 128):
    scoresT[k, q] = KT_h[:, kc].T @ QT_h[:, qm]   (two heads row-packed, K=64)
    expT = exp(0.125 * scoresT)                   (ACT, N=1024 per inst)
    ctxT[65, q] += V'_h[kc].T @ expT              (row 64 = softmax denominator)
  epilogue: ctxT -> PE transpose -> [q, 65]; divide by denom; DMA out.
"""

import sys
import types

sys.path.insert(0, "/opt/trn_rl_repo")

import numpy as np

import concourse.bass as bass
import concourse.tile as tile
from concourse import bacc, mybir
from concourse.bass_utils import run_bass_kernel_spmd
from concourse.masks import make_identity

B, S, H = 4, 2048, 1024
NH, HD = 16, 64
NCORES = 8
HEADS_PER_CORE = NH // 2      # 8 heads per core
HG = HEADS_PER_CORE * HD      # 512 = per-core head width
P = 128
QM = 512                      # q macro-tile
N_QM = S // QM                # 4
N_KC = S // P                 # 16 k chunks
N_ST = S // P                 # 16 s tiles
N_HB = H // P                 # 8 h chunks (contraction)
N_MT = HG // P                # 4 hd m-tiles

FP32 = mybir.dt.float32
BF16 = mybir.dt.bfloat16


def _ensure_profile_hook():
    """The image's antenv lacks axon_hooks; shim it so trace=True works."""
    try:
        from antenv.axon_hooks import get_axon_ntff_profile_hook  # noqa: F401
        return
    except ImportError:
        pass
    try:
        from trn_agent_boot.trn_boot import _ntff_profile_via_ctypes
    except ImportError:
        return
    hook = _ntff_profile_via_ctypes("/opt/axon/libaxon_pjrt.so")
    mod = types.ModuleType("antenv.axon_hooks")
    mod.get_axon_ntff_profile_hook = lambda: hook
    mod.set_axon_ntff_profile_hook = lambda h: None
    sys.modules["antenv.axon_hooks"] = mod


def build():
    nc = bacc.Bacc("TRN2", target_bir_lowering=False, debug=False,
                   num_devices=NCORES)

    x_d = nc.declare_dram_parameter("x", [S, H], FP32, isOutput=False)
    wq_d = nc.declare_dram_parameter("wq", [H, HG], FP32, isOutput=False)
    wk_d = nc.declare_dram_parameter("wk", [H, HG], FP32, isOutput=False)
    wv_d = nc.declare_dram_parameter("wv", [H, HG], FP32, isOutput=False)
    bq_d = nc.declare_dram_parameter("bq", [HG], FP32, isOutput=False)
    bk_d = nc.declare_dram_parameter("bk", [HG], FP32, isOutput=False)
    bv_d = nc.declare_dram_parameter("bv", [HG], FP32, isOutput=False)
    out_d = nc.declare_dram_parameter("out", [S, HG], FP32, isOutput=True)

    with tile.TileContext(nc) as tc:
        _build_body(nc, tc, x_d, (wq_d, wk_d, wv_d), (bq_d, bk_d, bv_d), out_d)

    nc.finalize()
    return nc


def _build_body(nc, tc, x_d, w_d, b_d, out_d):
    wq_d, wk_d, wv_d = w_d
    bq_d, bk_d, bv_d = b_d

    import contextlib
    ctx = contextlib.ExitStack()
    with ctx:
        const = ctx.enter_context(tc.tile_pool(name="const", bufs=1))
        xf = ctx.enter_context(tc.tile_pool(name="xf", bufs=N_ST))
        big = ctx.enter_context(tc.tile_pool(name="big", bufs=1))
        wstage = ctx.enter_context(tc.tile_pool(name="wstage", bufs=3))
        expp = ctx.enter_context(tc.tile_pool(name="expp", bufs=4))
        epil = ctx.enter_context(tc.tile_pool(name="epil", bufs=3))
        outp = ctx.enter_context(tc.tile_pool(name="outp", bufs=8))
        ps_sc = ctx.enter_context(
            tc.tile_pool(name="ps_sc", bufs=2, space="PSUM"))
        ps_ctx = ctx.enter_context(
            tc.tile_pool(name="ps_ctx", bufs=2, space="PSUM"))
        ps_misc = ctx.enter_context(
            tc.tile_pool(name="ps_misc", bufs=2, space="PSUM"))

        # ---- constants -------------------------------------------------
        ident_f = const.tile([P, P], FP32)
        make_identity(nc, ident_f)
        ident_b = const.tile([P, P], BF16)
        make_identity(nc, ident_b)

        bqT = const.tile([P, N_MT], FP32)
        nc.sync.dma_start(out=bqT, in_=bq_d.ap().rearrange("(o p) -> p o", p=P))
        bkT = const.tile([P, N_MT], FP32)
        nc.sync.dma_start(out=bkT, in_=bk_d.ap().rearrange("(o p) -> p o", p=P))
        bv_ap = bv_d.ap()
        bvb = const.tile([P, HG], FP32)
        nc.sync.dma_start(
            out=bvb,
            in_=bass.AP(tensor=bv_ap.tensor, offset=bv_ap.offset,
                        ap=[[0, P]] + [list(a) for a in bv_ap.ap]),
        )

        # ---- weights: load f32, cast to bf16 ---------------------------
        w_sb = {}
        for name, wd in (("q", wq_d), ("k", wk_d), ("v", wv_d)):
            wt = big.tile([P, N_HB, HG], BF16, tag=f"w{name}")
            for k in range(N_HB):
                stg = wstage.tile([P, HG], FP32, tag="wstg")
                nc.sync.dma_start(out=stg, in_=wd.ap()[k * P:(k + 1) * P, :])
                nc.vector.tensor_copy(out=wt[:, k, :], in_=stg)
            w_sb[name] = wt

        # ---- x load + transpose to xT (bf16) ---------------------------
        x_sb = []
        for st in range(N_ST):
            xt = xf.tile([P, H], FP32, tag="x")
            nc.sync.dma_start(out=xt, in_=x_d.ap()[st * P:(st + 1) * P, :])
            x_sb.append(xt)

        xT = big.tile([P, N_HB, S], BF16, tag="xT")
        for hb in range(N_HB):
            for sq in range(4):
                ps = ps_misc.tile([P, 4, P], FP32, tag="misc")
                for q in range(4):
                    nc.tensor.transpose(
                        ps[:, q, :],
                        x_sb[sq * 4 + q][:, hb * P:(hb + 1) * P],
                        ident_f,
                    )
                nc.vector.tensor_copy(
                    out=xT[:, hb, sq * QM:(sq + 1) * QM],
                    in_=ps.rearrange("p a b -> p (a b)"),
                )

        # ---- V' projection: [128s, 8h, 65] with ones column ------------
        vp = big.tile([P, N_ST, HEADS_PER_CORE, HD + 1], BF16, tag="vp")
        nc.vector.memset(vp, 1.0)
        for st in range(N_ST):
            ps = ps_misc.tile([P, HG], FP32, tag="misc")
            for hb in range(N_HB):
                nc.tensor.matmul(
                    ps,
                    lhsT=xT[:, hb, st * P:(st + 1) * P],
                    rhs=w_sb["v"][:, hb, :],
                    start=(hb == 0),
                    stop=(hb == N_HB - 1),
                )
            nc.vector.scalar_tensor_tensor(
                out=vp[:, st, :, 0:HD],
                in0=ps.rearrange("p (h d) -> p h d", h=HEADS_PER_CORE),
                scalar=1.0,
                in1=bvb.rearrange("p (h d) -> p h d", h=HEADS_PER_CORE),
                op0=mybir.AluOpType.mult,
                op1=mybir.AluOpType.add,
            )

        # ---- Q/K projections (transposed layout) -----------------------
        qT = big.tile([P, N_MT, S], BF16, tag="qT")
        kT = big.tile([P, N_MT, S], BF16, tag="kT")

        def proj(mt):
            for w_name, dst, bias in (("q", qT, bqT), ("k", kT, bkT)):
                for n in range(N_QM):
                    ps = ps_misc.tile([P, QM], FP32, tag="misc")
                    for k in range(N_HB):
                        nc.tensor.matmul(
                            ps,
                            lhsT=w_sb[w_name][:, k, mt * P:(mt + 1) * P],
                            rhs=xT[:, k, n * QM:(n + 1) * QM],
                            start=(k == 0),
                            stop=(k == N_HB - 1),
                        )
                    nc.vector.tensor_scalar_add(
                        out=dst[:, mt, n * QM:(n + 1) * QM],
                        in0=ps,
                        scalar1=bias[:, mt:mt + 1],
                    )

        # ---- attention for one head pair -------------------------------
        def attention(hp):
            for qm in range(N_QM):
                ctx_ps = [ps_ctx.tile([HD + 1, QM], FP32, tag="ctx",
                                      name=f"ctx{hh}")
                          for hh in range(2)]
                for kc in range(N_KC):
                    sc = ps_sc.tile([P, 2, QM], FP32, tag="sc")
                    for hh in range(2):
                        lo = hh * HD
                        nc.tensor.matmul(
                            sc[:, hh, :],
                            lhsT=kT[lo:lo + HD, hp, kc * P:(kc + 1) * P],
                            rhs=qT[lo:lo + HD, hp, qm * QM:(qm + 1) * QM],
                            start=True,
                            stop=True,
                            tile_position=(lo, 0),
                        )
                    et = expp.tile([P, 2, QM], BF16, tag="exp")
                    nc.scalar.activation(
                        out=et, in_=sc,
                        func=mybir.ActivationFunctionType.Exp,
                        scale=0.125,
                    )
                    for hh in range(2):
                        nc.tensor.matmul(
                            ctx_ps[hh],
                            lhsT=vp[:, kc, 2 * hp + hh, :],
                            rhs=et[:, hh, :],
                            start=(kc == 0),
                            stop=(kc == N_KC - 1),
                        )
                # epilogue: transpose ctxT back, divide by denominator
                for hh in range(2):
                    csb = epil.tile([HD + 1, QM], FP32, tag="ctxsb")
                    nc.vector.tensor_copy(out=csb, in_=ctx_ps[hh])
                    for qs in range(QM // P):
                        tp = ps_misc.tile([P, HD + 1], FP32, tag="misc")
                        nc.tensor.transpose(
                            tp,
                            csb[:, qs * P:(qs + 1) * P],
                            ident_f[0:HD + 1, 0:HD + 1],
                        )
                        rc = outp.tile([P, 1], FP32, tag="recip")
                        nc.vector.reciprocal(out=rc, in_=tp[:, HD:HD + 1])
                        ot = outp.tile([P, HD], FP32, tag="out")
                        nc.vector.tensor_scalar_mul(ot, tp[:, 0:HD], rc)
                        row = qm * QM + qs * P
                        col = (2 * hp + hh) * HD
                        nc.sync.dma_start(
                            out=out_d.ap()[row:row + P, col:col + HD],
                            in_=ot,
                        )

        proj(0)
        for hp in range(N_MT):
            attention(hp)
            if hp + 1 < N_MT:
                proj(hp + 1)


_NC_CACHE = None


def _get_nc():
    global _NC_CACHE
    if _NC_CACHE is None:
        _NC_CACHE = build()
    return _NC_CACHE


def make_in_maps(hidden_states, Wq, bq, Wk, bk, Wv, bv):
    hs = np.ascontiguousarray(np.asarray(hidden_states, dtype=np.float32))
    ws = {k: np.asarray(v, dtype=np.float32)
          for k, v in (("q", Wq), ("k", Wk), ("v", Wv))}
    bs = {k: np.asarray(v, dtype=np.float32)
          for k, v in (("q", bq), ("k", bk), ("v", bv))}
    in_maps = []
    for c in range(NCORES):
        b, g = c // 2, c % 2
        sl = slice(g * HG, (g + 1) * HG)
        in_maps.append({
            "x": np.ascontiguousarray(hs[b]),
            "wq": np.ascontiguousarray(ws["q"][:, sl]),
            "wk": np.ascontiguousarray(ws["k"][:, sl]),
            "wv": np.ascontiguousarray(ws["v"][:, sl]),
            "bq": np.ascontiguousarray(bs["q"][sl]),
            "bk": np.ascontiguousarray(bs["k"][sl]),
            "bv": np.ascontiguousarray(bs["v"][sl]),
        })
    return in_maps


def run(in_maps, trace=False):
    _ensure_profile_hook()
    nc = _get_nc()
    return run_bass_kernel_spmd(nc, in_maps, list(range(NCORES)), trace=trace)


def kernel(hidden_states, Wq, bq, Wk, bk, Wv, bv):
    in_maps = make_in_maps(hidden_states, Wq, bq, Wk, bk, Wv, bv)
    res = run(in_maps, trace=False)
    out = np.empty((B, S, H), dtype=np.float32)
    for c in range(NCORES):
        b, g = c // 2, c % 2
        out[b, :, g * HG:(g + 1) * HG] = res.results[c]["out"]
    return out


# revision 8
# speedup vs baseline: 1.0616x; 1.0616x over previous
"""BERT self-attention (B=4, S=2048, H=1024, 16 heads x 64) on 8 TRN2 NeuronCores.

Sharding: data-parallel over batch (4) x tensor-parallel over head-groups (2).
Core c handles batch c//2 and heads [8*(c%2), 8*(c%2)+8): it gets the full
hidden_states[b] plus the 512 W-columns/bias entries for its heads, and
produces out[b, :, 512*g : 512*(g+1)]. No cross-core communication.

Per-core kernel (all matmuls bf16, f32 accumulation in PSUM):
  xT   = transpose(x) via PE                      [1024h, 2048s]
  QT/KT = W.T @ xT  (+bias)                       [512hd, 2048s]
  V'   = xT.T @ Wv (+bias), 65-col per head with an appended ones column
  per (head-pair, q-macro 512, k-chunk 128):
    scoresT[k, q] = KT_h[:, kc].T @ QT_h[:, qm]   (two heads row-packed, K=64)
    expT = exp(0.125 * scoresT)                   (ACT, N=1024 per inst)
    ctxT[65, q] += V'_h[kc].T @ expT              (row 64 = softmax denominator)
  epilogue: ctxT -> PE transpose -> [q, 65]; divide by denom; DMA out.
"""

import sys
import types

sys.path.insert(0, "/opt/trn_rl_repo")

import numpy as np

import concourse.bass as bass
import concourse.tile as tile
from concourse import bacc, mybir
from concourse.bass_utils import run_bass_kernel_spmd
from concourse.masks import make_identity

B, S, H = 4, 2048, 1024
NH, HD = 16, 64
NCORES = 8
HEADS_PER_CORE = NH // 2      # 8 heads per core
HG = HEADS_PER_CORE * HD      # 512 = per-core head width
P = 128
QM = 512                      # q macro-tile
N_QM = S // QM                # 4
N_KC = S // P                 # 16 k chunks
N_ST = S // P                 # 16 s tiles
N_HB = H // P                 # 8 h chunks (contraction)
N_MT = HG // P                # 4 hd m-tiles

FP32 = mybir.dt.float32
BF16 = mybir.dt.bfloat16


def _ensure_profile_hook():
    """The image's antenv lacks axon_hooks; shim it so trace=True works."""
    try:
        from antenv.axon_hooks import get_axon_ntff_profile_hook  # noqa: F401
        return
    except ImportError:
        pass
    try:
        from trn_agent_boot.trn_boot import _ntff_profile_via_ctypes
    except ImportError:
        return
    hook = _ntff_profile_via_ctypes("/opt/axon/libaxon_pjrt.so")
    mod = types.ModuleType("antenv.axon_hooks")
    mod.get_axon_ntff_profile_hook = lambda: hook
    mod.set_axon_ntff_profile_hook = lambda h: None
    sys.modules["antenv.axon_hooks"] = mod


def build():
    nc = bacc.Bacc("TRN2", target_bir_lowering=False, debug=False,
                   num_devices=NCORES)

    x_d = nc.declare_dram_parameter("x", [S, H], FP32, isOutput=False)
    wq_d = nc.declare_dram_parameter("wq", [H, HG], FP32, isOutput=False)
    wk_d = nc.declare_dram_parameter("wk", [H, HG], FP32, isOutput=False)
    wv_d = nc.declare_dram_parameter("wv", [H, HG], FP32, isOutput=False)
    bq_d = nc.declare_dram_parameter("bq", [HG], FP32, isOutput=False)
    bk_d = nc.declare_dram_parameter("bk", [HG], FP32, isOutput=False)
    bv_d = nc.declare_dram_parameter("bv", [HG], FP32, isOutput=False)
    out_d = nc.declare_dram_parameter("out", [S, HG], FP32, isOutput=True)

    with tile.TileContext(nc) as tc:
        _build_body(nc, tc, x_d, (wq_d, wk_d, wv_d), (bq_d, bk_d, bv_d), out_d)

    nc.finalize()
    return nc


def _build_body(nc, tc, x_d, w_d, b_d, out_d):
    wq_d, wk_d, wv_d = w_d
    bq_d, bk_d, bv_d = b_d

    import contextlib
    ctx = contextlib.ExitStack()
    with ctx:
        const = ctx.enter_context(tc.tile_pool(name="const", bufs=1))
        xf = ctx.enter_context(tc.tile_pool(name="xf", bufs=6))
        big = ctx.enter_context(tc.tile_pool(name="big", bufs=1))
        wstage = ctx.enter_context(tc.tile_pool(name="wstage", bufs=3))
        expp = ctx.enter_context(tc.tile_pool(name="expp", bufs=4))
        epil = ctx.enter_context(tc.tile_pool(name="epil", bufs=3))
        outp = ctx.enter_context(tc.tile_pool(name="outp", bufs=8))
        ps_sc = ctx.enter_context(
            tc.tile_pool(name="ps_sc", bufs=2, space="PSUM"))
        ps_ctx = ctx.enter_context(
            tc.tile_pool(name="ps_ctx", bufs=2, space="PSUM"))
        ps_misc = ctx.enter_context(
            tc.tile_pool(name="ps_misc", bufs=2, space="PSUM"))

        # ---- constants -------------------------------------------------
        ident_f = const.tile([P, P], FP32)
        make_identity(nc, ident_f)
        ident_b = const.tile([P, P], BF16)
        make_identity(nc, ident_b)

        bqT = const.tile([P, N_MT], FP32)
        nc.sync.dma_start(out=bqT, in_=bq_d.ap().rearrange("(o p) -> p o", p=P))
        bkT = const.tile([P, N_MT], FP32)
        nc.sync.dma_start(out=bkT, in_=bk_d.ap().rearrange("(o p) -> p o", p=P))
        bv_ap = bv_d.ap()
        bvb = const.tile([P, HG], FP32)
        nc.sync.dma_start(
            out=bvb,
            in_=bass.AP(tensor=bv_ap.tensor, offset=bv_ap.offset,
                        ap=[[0, P]] + [list(a) for a in bv_ap.ap]),
        )

        # ---- weights: load f32 (gpsimd DMA queues), cast on idle ACT ---
        w_sb = {}
        for name, wd in (("q", wq_d), ("k", wk_d), ("v", wv_d)):
            wt = big.tile([P, N_HB, HG], BF16, tag=f"w{name}")
            for k in range(N_HB):
                stg = wstage.tile([P, HG], FP32, tag="wstg")
                nc.gpsimd.dma_start(out=stg, in_=wd.ap()[k * P:(k + 1) * P, :])
                nc.scalar.copy(out=wt[:, k, :], in_=stg)
            w_sb[name] = wt

        # ---- x load + transpose to xT (bf16), pipelined per s-quad -----
        xT = big.tile([P, N_HB, S], BF16, tag="xT")
        for sq in range(4):
            x_sb = []
            for q in range(4):
                st = sq * 4 + q
                xt = xf.tile([P, H], FP32, tag="x", name=f"x{st}")
                nc.sync.dma_start(out=xt, in_=x_d.ap()[st * P:(st + 1) * P, :])
                x_sb.append(xt)
            for hb in range(N_HB):
                ps = ps_misc.tile([P, 4, P], FP32, tag="misc")
                for q in range(4):
                    nc.tensor.transpose(
                        ps[:, q, :],
                        x_sb[q][:, hb * P:(hb + 1) * P],
                        ident_f,
                    )
                nc.vector.tensor_copy(
                    out=xT[:, hb, sq * QM:(sq + 1) * QM],
                    in_=ps.rearrange("p a b -> p (a b)"),
                )

        # ---- V' projection: [128s, 8h, 65] with ones column ------------
        vp = big.tile([P, N_ST, HEADS_PER_CORE, HD + 1], BF16, tag="vp")
        nc.vector.memset(vp, 1.0)
        for st in range(N_ST):
            ps = ps_misc.tile([P, HG], FP32, tag="misc")
            for hb in range(N_HB):
                nc.tensor.matmul(
                    ps,
                    lhsT=xT[:, hb, st * P:(st + 1) * P],
                    rhs=w_sb["v"][:, hb, :],
                    start=(hb == 0),
                    stop=(hb == N_HB - 1),
                )
            nc.vector.scalar_tensor_tensor(
                out=vp[:, st, :, 0:HD],
                in0=ps.rearrange("p (h d) -> p h d", h=HEADS_PER_CORE),
                scalar=1.0,
                in1=bvb.rearrange("p (h d) -> p h d", h=HEADS_PER_CORE),
                op0=mybir.AluOpType.mult,
                op1=mybir.AluOpType.add,
            )

        # ---- Q/K projections (transposed layout) -----------------------
        qT = big.tile([P, N_MT, S], BF16, tag="qT")
        kT = big.tile([P, N_MT, S], BF16, tag="kT")

        def proj_chunk(mt, n):
            for w_name, dst, bias in (("q", qT, bqT), ("k", kT, bkT)):
                ps = ps_misc.tile([P, QM], FP32, tag="misc")
                for k in range(N_HB):
                    nc.tensor.matmul(
                        ps,
                        lhsT=w_sb[w_name][:, k, mt * P:(mt + 1) * P],
                        rhs=xT[:, k, n * QM:(n + 1) * QM],
                        start=(k == 0),
                        stop=(k == N_HB - 1),
                    )
                nc.vector.tensor_scalar_add(
                    out=dst[:, mt, n * QM:(n + 1) * QM],
                    in0=ps,
                    scalar1=bias[:, mt:mt + 1],
                )

        # ---- attention for one head pair -------------------------------
        def attention(hp):
            for qm in range(N_QM):
                if hp + 1 < N_MT:
                    proj_chunk(hp + 1, qm)
                ctx_ps = [ps_ctx.tile([HD + 1, QM], FP32, tag="ctx",
                                      name=f"ctx{hh}")
                          for hh in range(2)]
                for kc in range(N_KC):
                    sc = ps_sc.tile([P, 2, QM], FP32, tag="sc")
                    for hh in range(2):
                        lo = hh * HD
                        nc.tensor.matmul(
                            sc[:, hh, :],
                            lhsT=kT[lo:lo + HD, hp, kc * P:(kc + 1) * P],
                            rhs=qT[lo:lo + HD, hp, qm * QM:(qm + 1) * QM],
                            start=True,
                            stop=True,
                            tile_position=(lo, 0),
                        )
                    et = expp.tile([P, 2, QM], BF16, tag="exp")
                    nc.scalar.activation(
                        out=et, in_=sc,
                        func=mybir.ActivationFunctionType.Exp,
                        scale=0.125,
                    )
                    for hh in range(2):
                        nc.tensor.matmul(
                            ctx_ps[hh],
                            lhsT=vp[:, kc, 2 * hp + hh, :],
                            rhs=et[:, hh, :],
                            start=(kc == 0),
                            stop=(kc == N_KC - 1),
                        )
                # epilogue: transpose ctxT back, divide by denominator
                for hh in range(2):
                    csb = epil.tile([HD + 1, QM], FP32, tag="ctxsb")
                    nc.vector.tensor_copy(out=csb, in_=ctx_ps[hh])
                    for qs in range(QM // P):
                        tp = ps_misc.tile([P, HD + 1], FP32, tag="misc")
                        nc.tensor.transpose(
                            tp,
                            csb[:, qs * P:(qs + 1) * P],
                            ident_f[0:HD + 1, 0:HD + 1],
                        )
                        rc = outp.tile([P, 1], FP32, tag="recip")
                        nc.vector.reciprocal(out=rc, in_=tp[:, HD:HD + 1])
                        ot = outp.tile([P, HD], FP32, tag="out")
                        nc.vector.tensor_scalar_mul(ot, tp[:, 0:HD], rc)
                        row = qm * QM + qs * P
                        col = (2 * hp + hh) * HD
                        nc.sync.dma_start(
                            out=out_d.ap()[row:row + P, col:col + HD],
                            in_=ot,
                        )

        for n in range(N_QM):
            proj_chunk(0, n)
        for hp in range(N_MT):
            attention(hp)


_NC_CACHE = None


def _get_nc():
    global _NC_CACHE
    if _NC_CACHE is None:
        _NC_CACHE = build()
    return _NC_CACHE


def make_in_maps(hidden_states, Wq, bq, Wk, bk, Wv, bv):
    hs = np.ascontiguousarray(np.asarray(hidden_states, dtype=np.float32))
    ws = {k: np.asarray(v, dtype=np.float32)
          for k, v in (("q", Wq), ("k", Wk), ("v", Wv))}
    bs = {k: np.asarray(v, dtype=np.float32)
          for k, v in (("q", bq), ("k", bk), ("v", bv))}
    in_maps = []
    for c in range(NCORES):
        b, g = c // 2, c % 2
        sl = slice(g * HG, (g + 1) * HG)
        in_maps.append({
            "x": np.ascontiguousarray(hs[b]),
            "wq": np.ascontiguousarray(ws["q"][:, sl]),
            "wk": np.ascontiguousarray(ws["k"][:, sl]),
            "wv": np.ascontiguousarray(ws["v"][:, sl]),
            "bq": np.ascontiguousarray(bs["q"][sl]),
            "bk": np.ascontiguousarray(bs["k"][sl]),
            "bv": np.ascontiguousarray(bs["v"][sl]),
        })
    return in_maps


def run(in_maps, trace=False):
    _ensure_profile_hook()
    nc = _get_nc()
    return run_bass_kernel_spmd(nc, in_maps, list(range(NCORES)), trace=trace)


def kernel(hidden_states, Wq, bq, Wk, bk, Wv, bv):
    in_maps = make_in_maps(hidden_states, Wq, bq, Wk, bk, Wv, bv)
    res = run(in_maps, trace=False)
    out = np.empty((B, S, H), dtype=np.float32)
    for c in range(NCORES):
        b, g = c // 2, c % 2
        out[b, :, g * HG:(g + 1) * HG] = res.results[c]["out"]
    return out


# revision 12
# speedup vs baseline: 1.1141x; 1.0495x over previous
"""BERT self-attention (B=4, S=2048, H=1024, 16 heads x 64) on 8 TRN2 NeuronCores.

Sharding: data-parallel over batch (4) x tensor-parallel over head-groups (2).
Core c handles batch c//2 and heads [8*(c%2), 8*(c%2)+8): it gets the full
hidden_states[b] plus the 512 W-columns/bias entries for its heads, and
produces out[b, :, 512*g : 512*(g+1)]. No cross-core communication.

Per-core kernel (all matmuls bf16, f32 accumulation in PSUM):
  xT   = transpose(x) via PE                      [1024h, 2048s]
  QT/KT = W.T @ xT  (+bias)                       [512hd, 2048s]
  V'   = xT.T @ Wv (+bias), 65-col per head with an appended ones column
  per (head-pair, q-macro 512, k-chunk 128):
    scoresT[k, q] = KT_h[:, kc].T @ QT_h[:, qm]   (two heads row-packed, K=64)
    expT = exp(0.125 * scoresT)                   (ACT, N=1024 per inst)
    ctxT[65, q] += V'_h[kc].T @ expT              (row 64 = softmax denominator)
  epilogue: ctxT -> PE transpose -> [q, 65]; divide by denom; DMA out.
"""

import sys
import types

sys.path.insert(0, "/opt/trn_rl_repo")

import numpy as np

import concourse.bass as bass
import concourse.tile as tile
from concourse import bacc, mybir
from concourse.bass_utils import run_bass_kernel_spmd
from concourse.masks import make_identity

B, S, H = 4, 2048, 1024
NH, HD = 16, 64
NCORES = 8
HEADS_PER_CORE = NH // 2      # 8 heads per core
HG = HEADS_PER_CORE * HD      # 512 = per-core head width
P = 128
QM = 512                      # q macro-tile
N_QM = S // QM                # 4
N_KC = S // P                 # 16 k chunks
N_ST = S // P                 # 16 s tiles
N_HB = H // P                 # 8 h chunks (contraction)
N_MT = HG // P                # 4 hd m-tiles

FP32 = mybir.dt.float32
BF16 = mybir.dt.bfloat16


def _ensure_profile_hook():
    """The image's antenv lacks axon_hooks; shim it so trace=True works."""
    try:
        from antenv.axon_hooks import get_axon_ntff_profile_hook  # noqa: F401
        return
    except ImportError:
        pass
    try:
        from trn_agent_boot.trn_boot import _ntff_profile_via_ctypes
    except ImportError:
        return
    hook = _ntff_profile_via_ctypes("/opt/axon/libaxon_pjrt.so")
    mod = types.ModuleType("antenv.axon_hooks")
    mod.get_axon_ntff_profile_hook = lambda: hook
    mod.set_axon_ntff_profile_hook = lambda h: None
    sys.modules["antenv.axon_hooks"] = mod


def build():
    nc = bacc.Bacc("TRN2", target_bir_lowering=False, debug=False,
                   num_devices=NCORES)

    x_d = nc.declare_dram_parameter("x", [S, H], FP32, isOutput=False)
    wq_d = nc.declare_dram_parameter("wq", [H, HG], FP32, isOutput=False)
    wk_d = nc.declare_dram_parameter("wk", [H, HG], FP32, isOutput=False)
    wv_d = nc.declare_dram_parameter("wv", [H, HG], FP32, isOutput=False)
    bq_d = nc.declare_dram_parameter("bq", [HG], FP32, isOutput=False)
    bk_d = nc.declare_dram_parameter("bk", [HG], FP32, isOutput=False)
    bv_d = nc.declare_dram_parameter("bv", [HG], FP32, isOutput=False)
    out_d = nc.declare_dram_parameter("out", [S, HG], FP32, isOutput=True)

    with tile.TileContext(nc) as tc:
        _build_body(nc, tc, x_d, (wq_d, wk_d, wv_d), (bq_d, bk_d, bv_d), out_d)

    nc.finalize()
    return nc


def _build_body(nc, tc, x_d, w_d, b_d, out_d):
    wq_d, wk_d, wv_d = w_d
    bq_d, bk_d, bv_d = b_d

    import contextlib
    ctx = contextlib.ExitStack()
    with ctx:
        const = ctx.enter_context(tc.tile_pool(name="const", bufs=1))
        xf = ctx.enter_context(tc.tile_pool(name="xf", bufs=4))
        big = ctx.enter_context(tc.tile_pool(name="big", bufs=1))
        wstage = ctx.enter_context(tc.tile_pool(name="wstage", bufs=3))
        expp = ctx.enter_context(tc.tile_pool(name="expp", bufs=6))
        epil = ctx.enter_context(tc.tile_pool(name="epil", bufs=3))
        outp = ctx.enter_context(tc.tile_pool(name="outp", bufs=8))
        # PSUM budget (8 banks): ps_sc = 3 x 2-bank slots (scores double/
        # triple buffer; phase-1 transpose batches borrow a slot), ps_ctx =
        # 2 x 1-bank slots (ctx accumulators; phase-1 proj/V psums and
        # epilogue transpose batches borrow them when accumulators are free).
        ps_sc = ctx.enter_context(
            tc.tile_pool(name="ps_sc", bufs=3, space="PSUM"))
        ps_ctx = ctx.enter_context(
            tc.tile_pool(name="ps_ctx", bufs=2, space="PSUM"))

        # ---- constants -------------------------------------------------
        ident_f = const.tile([P, P], FP32)
        make_identity(nc, ident_f)
        ident_b = const.tile([P, P], BF16)
        make_identity(nc, ident_b)

        bqT = const.tile([P, N_MT], FP32)
        nc.sync.dma_start(out=bqT, in_=bq_d.ap().rearrange("(o p) -> p o", p=P))
        bkT = const.tile([P, N_MT], FP32)
        nc.sync.dma_start(out=bkT, in_=bk_d.ap().rearrange("(o p) -> p o", p=P))
        bv_ap = bv_d.ap()
        bvb = const.tile([P, HG], FP32)
        nc.sync.dma_start(
            out=bvb,
            in_=bass.AP(tensor=bv_ap.tensor, offset=bv_ap.offset,
                        ap=[[0, P]] + [list(a) for a in bv_ap.ap]),
        )

        # ---- weights: wv first (V' projection is on the critical path),
        # then wq/wk, on gpsimd DMA queues; casts on the still-idle ACT.
        w_sb = {}
        for name, wd in (("q", wq_d), ("k", wk_d), ("v", wv_d)):
            w_sb[name] = big.tile([P, N_HB, HG], BF16, tag=f"w{name}",
                                  name=f"w{name}")

        def load_w(name, wd):
            for k in range(N_HB):
                stg = wstage.tile([P, HG], FP32, tag="wstg", name=f"w{name}{k}")
                nc.gpsimd.dma_start(out=stg, in_=wd.ap()[k * P:(k + 1) * P, :])
                nc.scalar.copy(out=w_sb[name][:, k, :], in_=stg)

        load_w("v", wv_d)

        # ---- per s-tile: load x, transpose to xT, project V' -----------
        xT = big.tile([P, N_HB, S], BF16, tag="xT")
        vp = big.tile([P, N_ST, HEADS_PER_CORE, HD + 1], BF16, tag="vp")
        nc.vector.memset(vp, 1.0)

        qT = big.tile([P, N_MT, S], BF16, tag="qT")
        kT = big.tile([P, N_MT, S], BF16, tag="kT")

        def proj_chunk(mt, n):
            for w_name, dst, bias in (("q", qT, bqT), ("k", kT, bkT)):
                ps = ps_ctx.tile([P, QM], FP32, tag="ctx",
                                 name=f"proj{w_name}{mt}{n}")
                for k in range(N_HB):
                    nc.tensor.matmul(
                        ps,
                        lhsT=w_sb[w_name][:, k, mt * P:(mt + 1) * P],
                        rhs=xT[:, k, n * QM:(n + 1) * QM],
                        start=(k == 0),
                        stop=(k == N_HB - 1),
                    )
                nc.vector.tensor_scalar_add(
                    out=dst[:, mt, n * QM:(n + 1) * QM],
                    in0=ps,
                    scalar1=bias[:, mt:mt + 1],
                )

        for st in range(N_ST):
            if st == 1:
                load_w("q", wq_d)
                load_w("k", wk_d)
            xt = xf.tile([P, H], FP32, tag="x", name=f"x{st}")
            nc.sync.dma_start(out=xt, in_=x_d.ap()[st * P:(st + 1) * P, :])
            for half in range(2):
                ps = ps_sc.tile([P, 4, P], FP32, tag="sc", name=f"xt{st}{half}")
                for q in range(4):
                    hb = half * 4 + q
                    nc.tensor.transpose(
                        ps[:, q, :], xt[:, hb * P:(hb + 1) * P], ident_f)
                nc.vector.tensor_copy(
                    out=xT[:, half * 4:half * 4 + 4, st * P:(st + 1) * P],
                    in_=ps,
                )
            psv = ps_ctx.tile([P, HG], FP32, tag="ctx", name=f"v{st}")
            for hb in range(N_HB):
                nc.tensor.matmul(
                    psv,
                    lhsT=xT[:, hb, st * P:(st + 1) * P],
                    rhs=w_sb["v"][:, hb, :],
                    start=(hb == 0),
                    stop=(hb == N_HB - 1),
                )
            nc.vector.scalar_tensor_tensor(
                out=vp[:, st, :, 0:HD],
                in0=psv.rearrange("p (h d) -> p h d", h=HEADS_PER_CORE),
                scalar=1.0,
                in1=bvb.rearrange("p (h d) -> p h d", h=HEADS_PER_CORE),
                op0=mybir.AluOpType.mult,
                op1=mybir.AluOpType.add,
            )
            if st % 4 == 3:
                proj_chunk(0, st // 4)

        # ---- attention for one head pair -------------------------------
        def attention(hp):
            for qm in range(N_QM):
                ctx_ps = [ps_ctx.tile([HD + 1, QM], FP32, tag="ctx",
                                      name=f"ctx{hh}")
                          for hh in range(2)]
                for kc in range(N_KC):
                    sc = ps_sc.tile([P, 2, QM], FP32, tag="sc")
                    for hh in range(2):
                        lo = hh * HD
                        nc.tensor.matmul(
                            sc[:, hh, :],
                            lhsT=kT[lo:lo + HD, hp, kc * P:(kc + 1) * P],
                            rhs=qT[lo:lo + HD, hp, qm * QM:(qm + 1) * QM],
                            start=True,
                            stop=True,
                            tile_position=(lo, 0),
                        )
                    et = expp.tile([P, 2, QM], BF16, tag="exp")
                    nc.scalar.activation(
                        out=et, in_=sc,
                        func=mybir.ActivationFunctionType.Exp,
                        scale=0.125,
                    )
                    for hh in range(2):
                        nc.tensor.matmul(
                            ctx_ps[hh],
                            lhsT=vp[:, kc, 2 * hp + hh, :],
                            rhs=et[:, hh, :],
                            start=(kc == 0),
                            stop=(kc == N_KC - 1),
                        )
                # epilogue: transpose ctxT back, divide by denominator
                for hh in range(2):
                    csb = epil.tile([HD + 1, QM], FP32, tag="ctxsb")
                    nc.vector.tensor_copy(out=csb, in_=ctx_ps[hh])
                    tp = ps_ctx.tile([P, QM // P, HD + 1], FP32, tag="ctx",
                                     name=f"tp{hh}")
                    for qs in range(QM // P):
                        nc.tensor.transpose(
                            tp[:, qs, :],
                            csb[:, qs * P:(qs + 1) * P],
                            ident_f[0:HD + 1, 0:HD + 1],
                        )
                    for qs in range(QM // P):
                        rc = outp.tile([P, 1], FP32, tag="recip")
                        nc.vector.reciprocal(out=rc, in_=tp[:, qs, HD:HD + 1])
                        ot = outp.tile([P, HD], FP32, tag="out")
                        nc.vector.tensor_scalar_mul(ot, tp[:, qs, 0:HD], rc)
                        row = qm * QM + qs * P
                        col = (2 * hp + hh) * HD
                        nc.sync.dma_start(
                            out=out_d.ap()[row:row + P, col:col + HD],
                            in_=ot,
                        )
                # next head-pair's Q/K projection chunk fills the PE gap
                if hp + 1 < N_MT:
                    proj_chunk(hp + 1, qm)

        for hp in range(N_MT):
            attention(hp)


_NC_CACHE = None


def _get_nc():
    global _NC_CACHE
    if _NC_CACHE is None:
        _NC_CACHE = build()
    return _NC_CACHE


def make_in_maps(hidden_states, Wq, bq, Wk, bk, Wv, bv):
    hs = np.ascontiguousarray(np.asarray(hidden_states, dtype=np.float32))
    ws = {k: np.asarray(v, dtype=np.float32)
          for k, v in (("q", Wq), ("k", Wk), ("v", Wv))}
    bs = {k: np.asarray(v, dtype=np.float32)
          for k, v in (("q", bq), ("k", bk), ("v", bv))}
    in_maps = []
    for c in range(NCORES):
        b, g = c // 2, c % 2
        sl = slice(g * HG, (g + 1) * HG)
        in_maps.append({
            "x": np.ascontiguousarray(hs[b]),
            "wq": np.ascontiguousarray(ws["q"][:, sl]),
            "wk": np.ascontiguousarray(ws["k"][:, sl]),
            "wv": np.ascontiguousarray(ws["v"][:, sl]),
            "bq": np.ascontiguousarray(bs["q"][sl]),
            "bk": np.ascontiguousarray(bs["k"][sl]),
            "bv": np.ascontiguousarray(bs["v"][sl]),
        })
    return in_maps


def run(in_maps, trace=False):
    _ensure_profile_hook()
    nc = _get_nc()
    return run_bass_kernel_spmd(nc, in_maps, list(range(NCORES)), trace=trace)


def kernel(hidden_states, Wq, bq, Wk, bk, Wv, bv):
    in_maps = make_in_maps(hidden_states, Wq, bq, Wk, bk, Wv, bv)
    res = run(in_maps, trace=False)
    out = np.empty((B, S, H), dtype=np.float32)
    for c in range(NCORES):
        b, g = c // 2, c % 2
        out[b, :, g * HG:(g + 1) * HG] = res.results[c]["out"]
    return out
